# revision 36
# baseline (speedup 1.0000x reference)
"""Bilinear interpolation (affine grid sampling) Trainium2 Bass kernel, v4.

image [32,256,256,32] f32 + theta [32,6] f32 -> out [32,256,256,32] f32.
Data-parallel over batch: 4 samples per core on 8 cores.

Pipeline per core (all engines busy, gather-bound):
  - Host builds per-sample bf16 corner tables qimg[y*128+u] = 2 rows x 4
    cols, COLUMN-major [col, row, ch], 512B stride. The device gathers
    only the first 3 column-pairs (384B) of each unit; col 3 pads the
    stride to a 256B multiple (ucode needs stride%256, not elem%256).
  - Coordinate pipelines (DVE, f32) compute int16 unit indices in the
    dma_gather wrapped layout plus six parity-folded weight planes in the
    out layout. The pixel->slot maps (_pix_out/_pix_wrapped) give each of
    the 16 SWDGE idx channels consecutive pixels, so each SDMA engine
    walks near-sequential HBM addresses.
  - Gathers: 4x 1024-idx dma_gather per 4096-pixel block, one per SWDGE
    queue. Queue q's descriptor generation runs on Q7 core pair (2q,2q+1)
    so desc-gen parallelizes 4x across the 8 GPSIMD cores; 1024 idx =
    64 descs/engine = exactly one packet (single_packet HW limit).
  - Blend: ACT materializes each weight plane into a packed bf16 tile
    (broadcast operands would break the DVE's fast bf16 mode), DVE does
    the 6 products, PE accumulates them into PSUM via identity matmuls,
    DVE/ACT alternate copying PSUM out as bf16.
  - Output stored bf16 [S, NCALL, P, KB, C]; host unpermutes + converts.
"""

import sys

sys.path.insert(0, "/opt/trn_rl_repo")

from contextlib import ExitStack

import numpy as np
import ml_dtypes

import concourse.bacc as bacc
import concourse.bass as bass
import concourse.tile as tile
from concourse import mybir
from concourse.bass_utils import run_bass_kernel_spmd
from concourse.library_config import mlp

B_TOTAL = 32
N_CORES = 8
S = B_TOTAL // N_CORES      # 4 samples per core
H = W = 256
C = 32
HW = H * W                  # 65536
ELEM = 256                  # 8 pixels of 32 channels bf16 = 512B gather element
NU = (W // 2) * H           # 32768 units per sample
P = 128
NIDX = 4096                 # indices per dma_gather call
NCALL = HW // NIDX          # 16 calls per sample
KB = NIDX // P              # 32 pixels per partition per call
Q = HW // P                 # 512 columns in out-layout coordinate tiles
FW = HW // 16 // 2          # 2048 wrapped-f columns (two partition halves)

BF16 = ml_dtypes.bfloat16

_COMPILED = {}


def _dma_gather_384(gp, out_ap, in_ap, idxs_ap, num_idxs, num_idxs_reg,
                    elem_size, elem_step, queue_num, single_packet=True):
    """Non-transpose HBM-source dma_gather without the elem%256B assert.

    Mirrors bass.BassGpSimd.dma_gather's non-transpose path: the 256B
    multiple requirement is a transpose-mode (Xbar) restriction; the
    non-transpose ucode emits one plain SDMA descriptor of elem_size bytes
    per index, any length. stride (elem_step bytes) must still be a 256B
    multiple for the stride_bytes_256 encoding.
    """
    gp._assert_queue_num(queue_num)
    assert idxs_ap.dtype == mybir.dt.int16
    assert in_ap.dtype == out_ap.dtype
    assert bass.ap_utils.ap_is_contiguous(in_ap.ap[1:])
    assert bass.ap_utils.ap_is_contiguous(out_ap.ap[1:])
    assert bass.ap_utils.ap_is_contiguous(idxs_ap.ap[1:])
    assert in_ap.ap[-1][1] == out_ap.ap[-1][1] == elem_size
    assert out_ap.ap[0][1] * out_ap.ap[1][1] == bass.round_up_to_multiple(
        num_idxs, 128)
    assert in_ap.ap[0][0] == elem_step
    stride_bytes = elem_step * mybir.dt.size(in_ap.dtype)
    stride_bytes_256 = bass.exact_div(stride_bytes, 256)
    assert stride_bytes_256 < 256

    _in_ap = gp.lower_ap_dma(in_ap, for_custom_bir_dma=True)
    _idxs_ap = gp.lower_ap(idxs_ap)
    _out_ap = gp.lower_ap(out_ap)
    return gp.add_instruction(
        mybir.InstDMAGatherAnt(
            name=gp.bass.get_next_instruction_name(),
            ins=[*_in_ap, _idxs_ap,
                 gp.lower_val_access(gp.to_reg(num_idxs_reg))],
            outs=[_out_ap],
            transpose=False,
            num_idxs=num_idxs,
            elem_size=elem_size,
            stride_bytes_256=stride_bytes_256,
            gen_mode=0,
            single_packet=single_packet,
            queue_num=queue_num,
            sbuf_tokens_per_rank=0,
            sbuf_free_dim_per_rank=0,
            sbuf_free_dim_pad_per_rank=0,
            sbuf_byte_offset=0,
        )
    )


def _build_nc(n_reps=1, skip_gather=False, skip_blend=False, big_calls=False):
    f32 = mybir.dt.float32
    bf16 = mybir.dt.bfloat16
    i32 = mybir.dt.int32
    i16 = mybir.dt.int16
    AF = mybir.AluOpType
    ACTF = mybir.ActivationFunctionType
    nc = bacc.Bacc("TRN2", target_bir_lowering=False,
                   dynamic_dma_scratch_size=16384, num_swdge_queues=4)

    qimg = [nc.dram_tensor(f"qimg{b}", [NU, ELEM], bf16, kind="ExternalInput")
            for b in range(S)]
    xg_o_d = nc.dram_tensor("xg_o", [P, Q], f32, kind="ExternalInput")
    yg_o_d = nc.dram_tensor("yg_o", [P, Q], f32, kind="ExternalInput")
    xg_w_d = nc.dram_tensor("xg_w", [P, FW], f32, kind="ExternalInput")
    yg_w_d = nc.dram_tensor("yg_w", [P, FW], f32, kind="ExternalInput")
    th_o_d = nc.dram_tensor("th_o", [P, 6 * S], f32, kind="ExternalInput")
    th_w_d = nc.dram_tensor("th_w", [P, 6], f32, kind="ExternalInput")
    id_d = nc.dram_tensor("id128", [P, P], bf16, kind="ExternalInput")
    out_d = nc.dram_tensor("out", [S, NCALL, P, KB, C], bf16,
                           kind="ExternalOutput")

    V = nc.vector
    ACT = nc.scalar

    with tile.TileContext(nc) as tc, ExitStack() as ctx:
        nc.gpsimd.load_library(mlp)

        singles = ctx.enter_context(tc.tile_pool(name="singles", bufs=1))
        idx16w = singles.tile([P, FW], i16)       # wrapped idx, all samples
        xg_o = singles.tile([P, Q], f32)
        yg_o = singles.tile([P, Q], f32)
        th_o = singles.tile([P, 6 * S], f32)
        th_w = singles.tile([P, 6], f32)
        id128 = singles.tile([P, P], bf16)
        nc.sync.dma_start(out=xg_o[:], in_=xg_o_d[:])
        nc.sync.dma_start(out=yg_o[:], in_=yg_o_d[:])
        nc.sync.dma_start(out=th_o[:], in_=th_o_d[:])
        nc.sync.dma_start(out=th_w[:], in_=th_w_d[:])
        nc.sync.dma_start(out=id128[:], in_=id_d[:])

        # ---- Phase W: wrapped int16 index pipeline (all samples at once) ----
        with ExitStack() as wctx:
            wpool = wctx.enter_context(tc.tile_pool(name="wpool", bufs=1))

            def wt(tag, dt=f32):
                return wpool.tile([P, FW], dt, tag=tag, name=tag)

            xgw = wt("xgw")
            ygw = wt("ygw")
            nc.sync.dma_start(out=xgw[:], in_=xg_w_d[:])
            nc.sync.dma_start(out=ygw[:], in_=yg_w_d[:])
            tw = [th_w[:, k:k + 1] for k in range(6)]

            def w_affine(t0, t1, t2, tag):
                a = wt("wA")
                b = wt("wB")
                V.tensor_scalar(out=a[:], in0=xgw[:], scalar1=t0,
                                scalar2=None, op0=AF.mult)
                V.tensor_scalar(out=b[:], in0=ygw[:], scalar1=t1,
                                scalar2=None, op0=AF.mult)
                v = wt(tag)
                V.tensor_tensor(out=v[:], in0=a[:], in1=b[:], op=AF.add)
                V.tensor_scalar(out=v[:], in0=v[:], scalar1=t2,
                                scalar2=None, op0=AF.add)
                V.tensor_scalar(out=v[:], in0=v[:], scalar1=1.0,
                                scalar2=128.0, op0=AF.add, op1=AF.mult)
                return v

            xw = w_affine(tw[0], tw[1], tw[2], "wX")
            yw = w_affine(tw[3], tw[4], tw[5], "wY")

            def w_trunc(v, dsttag):
                vi = wt("wI", i32)
                V.tensor_copy(out=vi[:], in_=v[:])
                vf = wt(dsttag)
                V.tensor_copy(out=vf[:], in_=vi[:])
                g = wt("wC")
                V.tensor_tensor(out=g[:], in0=vf[:], in1=v[:], op=AF.is_gt)
                l = wt("wD")
                V.tensor_tensor(out=l[:], in0=vf[:], in1=v[:], op=AF.is_lt)
                nn = wt("wE")
                V.tensor_scalar(out=nn[:], in0=v[:], scalar1=0.0,
                                scalar2=None, op0=AF.is_ge)
                V.tensor_tensor(out=g[:], in0=g[:], in1=nn[:], op=AF.mult)
                ng = wt("wE")
                V.tensor_scalar(out=ng[:], in0=v[:], scalar1=0.0,
                                scalar2=None, op0=AF.is_lt)
                V.tensor_tensor(out=l[:], in0=l[:], in1=ng[:], op=AF.mult)
                V.tensor_tensor(out=vf[:], in0=vf[:], in1=g[:], op=AF.subtract)
                V.tensor_tensor(out=vf[:], in0=vf[:], in1=l[:], op=AF.add)
                return vf

            x0fw = w_trunc(xw, "wA")       # xw's a/b scratch done
            y0fw = w_trunc(yw, "wB")
            V.tensor_scalar(out=x0fw[:], in0=x0fw[:], scalar1=float(W - 1),
                            scalar2=0.0, op0=AF.min, op1=AF.max)   # x0c
            V.tensor_scalar(out=y0fw[:], in0=y0fw[:], scalar1=float(H - 1),
                            scalar2=0.0, op0=AF.min, op1=AF.max)   # y0c
            xp = wt("wC")
            V.tensor_scalar(out=xp[:], in0=x0fw[:], scalar1=0.5,
                            scalar2=None, op0=AF.mult)
            xpi = wt("wI", i32)
            V.tensor_copy(out=xpi[:], in_=xp[:])
            xpf = wt("wD")
            V.tensor_copy(out=xpf[:], in_=xpi[:])
            g = wt("wE")
            V.tensor_tensor(out=g[:], in0=xpf[:], in1=xp[:], op=AF.is_gt)
            V.tensor_tensor(out=xpf[:], in0=xpf[:], in1=g[:], op=AF.subtract)
            idxf = wt("wX")
            V.tensor_scalar(out=idxf[:], in0=y0fw[:], scalar1=float(W // 2),
                            scalar2=None, op0=AF.mult)
            V.tensor_tensor(out=idxf[:], in0=idxf[:], in1=xpf[:], op=AF.add)
            V.tensor_copy(out=idx16w[:], in_=idxf[:])

        # ---- steady-state pools ----
        coord = ctx.enter_context(tc.tile_pool(name="coord", bufs=1))
        persist = ctx.enter_context(tc.tile_pool(name="persist", bufs=2))
        reps = ctx.enter_context(tc.tile_pool(name="reps", bufs=2))
        gpool = ctx.enter_context(tc.tile_pool(name="gpool", bufs=4))
        mpool = ctx.enter_context(tc.tile_pool(name="mpool", bufs=1))
        ppool = ctx.enter_context(tc.tile_pool(name="ppool", bufs=1))
        opool = ctx.enter_context(tc.tile_pool(name="opool", bufs=2))
        psum = ctx.enter_context(tc.tile_pool(name="psum", bufs=4,
                                              space="PSUM"))

        def ctile(tag, dt=f32):
            return coord.tile([P, Q], dt, tag=tag, name=tag)

        for b in [bb for _ in range(n_reps) for bb in range(S)]:
            t = [th_o[:, 6 * b + k: 6 * b + k + 1] for k in range(6)]

            def affine(t0, t1, t2, tag):
                a = ctile("scrA")
                bb = ctile("scrB")
                V.tensor_scalar(out=a[:], in0=xg_o[:], scalar1=t0,
                                scalar2=None, op0=AF.mult)
                V.tensor_scalar(out=bb[:], in0=yg_o[:], scalar1=t1,
                                scalar2=None, op0=AF.mult)
                v = ctile(tag)
                V.tensor_tensor(out=v[:], in0=a[:], in1=bb[:], op=AF.add)
                V.tensor_scalar(out=v[:], in0=v[:], scalar1=t2,
                                scalar2=None, op0=AF.add)
                V.tensor_scalar(out=v[:], in0=v[:], scalar1=1.0,
                                scalar2=128.0, op0=AF.add, op1=AF.mult)
                return v

            x = affine(t[0], t[1], t[2], "x")
            y = affine(t[3], t[4], t[5], "y")

            def trunc_f(v, tag):
                vi = ctile("scrI", i32)
                V.tensor_copy(out=vi[:], in_=v[:])
                vf = ctile(tag)
                V.tensor_copy(out=vf[:], in_=vi[:])
                g = ctile("scrC")
                V.tensor_tensor(out=g[:], in0=vf[:], in1=v[:], op=AF.is_gt)
                l = ctile("scrD")
                V.tensor_tensor(out=l[:], in0=vf[:], in1=v[:], op=AF.is_lt)
                nn = ctile("scrE")
                V.tensor_scalar(out=nn[:], in0=v[:], scalar1=0.0,
                                scalar2=None, op0=AF.is_ge)
                V.tensor_tensor(out=g[:], in0=g[:], in1=nn[:], op=AF.mult)
                ng = ctile("scrE")
                V.tensor_scalar(out=ng[:], in0=v[:], scalar1=0.0,
                                scalar2=None, op0=AF.is_lt)
                V.tensor_tensor(out=l[:], in0=l[:], in1=ng[:], op=AF.mult)
                V.tensor_tensor(out=vf[:], in0=vf[:], in1=g[:], op=AF.subtract)
                V.tensor_tensor(out=vf[:], in0=vf[:], in1=l[:], op=AF.add)
                return vf

            x0f = trunc_f(x, "x0f")
            y0f = trunc_f(y, "y0f")

            x0c = ctile("x0c")
            V.tensor_scalar(out=x0c[:], in0=x0f[:], scalar1=float(W - 1),
                            scalar2=0.0, op0=AF.min, op1=AF.max)
            y0c = ctile("y0c")
            V.tensor_scalar(out=y0c[:], in0=y0f[:], scalar1=float(H - 1),
                            scalar2=0.0, op0=AF.min, op1=AF.max)
            x1c = ctile("x1c")
            V.tensor_scalar(out=x1c[:], in0=x0f[:], scalar1=1.0,
                            scalar2=float(W - 1), op0=AF.add, op1=AF.min)
            V.tensor_scalar(out=x1c[:], in0=x1c[:], scalar1=0.0,
                            scalar2=None, op0=AF.max)
            y1c = ctile("y1c")
            V.tensor_scalar(out=y1c[:], in0=y0f[:], scalar1=1.0,
                            scalar2=float(H - 1), op0=AF.add, op1=AF.min)
            V.tensor_scalar(out=y1c[:], in0=y1c[:], scalar1=0.0,
                            scalar2=None, op0=AF.max)

            def tt_new(i0, i1, op, tag, pool=None):
                o = ctile(tag) if pool is None else pool.tile(
                    [P, Q], f32, tag=tag, name=tag)
                V.tensor_tensor(out=o[:], in0=i0[:], in1=i1[:], op=op)
                return o

            def tt_ip(dst, i1, op):
                V.tensor_tensor(out=dst[:], in0=dst[:], in1=i1[:], op=op)

            u1 = tt_new(x1c, x, AF.subtract, "u1")
            u0 = tt_new(x, x0c, AF.subtract, "u0")
            v1 = tt_new(y1c, y, AF.subtract, "v1")
            v0 = tt_new(y, y0c, AF.subtract, "v0")

            wa = tt_new(u1, v1, AF.mult, "wa")
            wb = tt_new(u1, v0, AF.mult, "wb")
            wc = tt_new(u0, v1, AF.mult, "wc")
            wd = tt_new(u0, v0, AF.mult, "wd")

            cx = tt_new(x1c, x0c, AF.is_equal, "cx")
            cy = tt_new(y1c, y0c, AF.is_equal, "cy")
            sx = ctile("scrA")
            V.tensor_scalar(out=sx[:], in0=cx[:], scalar1=-1.0,
                            scalar2=1.0, op0=AF.mult, op1=AF.add)
            sy = ctile("scrB")
            V.tensor_scalar(out=sy[:], in0=cy[:], scalar1=-1.0,
                            scalar2=1.0, op0=AF.mult, op1=AF.add)

            # y-fold then x-fold: move invalid-neighbor weights onto the
            # pixels actually fetched.
            ft = tt_new(wb, cy, AF.mult, "ft")
            tt_ip(wa, ft, AF.add)
            tt_ip(wb, sy, AF.mult)
            ft = tt_new(wd, cy, AF.mult, "ft")
            tt_ip(wc, ft, AF.add)
            tt_ip(wd, sy, AF.mult)
            ft = tt_new(wc, cx, AF.mult, "ft")
            tt_ip(wa, ft, AF.add)
            tt_ip(wc, sx, AF.mult)
            ft = tt_new(wd, cx, AF.mult, "ft")
            tt_ip(wb, ft, AF.add)
            tt_ip(wd, sx, AF.mult)

            # x-parity of x0c: par = x0c - 2*floor(x0c/2)
            xp = ctile("scrC")
            V.tensor_scalar(out=xp[:], in0=x0c[:], scalar1=0.5,
                            scalar2=None, op0=AF.mult)
            xpi = ctile("scrI", i32)
            V.tensor_copy(out=xpi[:], in_=xp[:])
            xpf = ctile("scrD")
            V.tensor_copy(out=xpf[:], in_=xpi[:])
            g = ctile("scrE")
            V.tensor_tensor(out=g[:], in0=xpf[:], in1=xp[:], op=AF.is_gt)
            V.tensor_tensor(out=xpf[:], in0=xpf[:], in1=g[:], op=AF.subtract)
            par = ctile("par")
            V.tensor_scalar(out=par[:], in0=xpf[:], scalar1=-2.0,
                            scalar2=None, op0=AF.mult)
            tt_ip(par, x0c, AF.add)
            sp = ctile("sp")
            V.tensor_scalar(out=sp[:], in0=par[:], scalar1=-1.0,
                            scalar2=1.0, op0=AF.mult, op1=AF.add)

            # six pre-multiplied weight planes (column-slice selection by parity)
            m0 = tt_new(wa, sp, AF.mult, "m0", persist)
            m1 = tt_new(wa, par, AF.mult, "m1", persist)
            ft = tt_new(wc, sp, AF.mult, "ft")
            tt_ip(m1, ft, AF.add)
            m2 = tt_new(wc, par, AF.mult, "m2", persist)
            m3 = tt_new(wb, sp, AF.mult, "m3", persist)
            m4 = tt_new(wb, par, AF.mult, "m4", persist)
            ft = tt_new(wd, sp, AF.mult, "ft")
            tt_ip(m4, ft, AF.add)
            m5 = tt_new(wd, par, AF.mult, "m5", persist)

            # replicate this sample's wrapped idx to all 8 Q7 groups
            idx_rep = reps.tile([P, 2 * FW], i16, tag="idx_rep", name="idx_rep")
            for g8 in range(8):
                nc.sync.dma_start(out=idx_rep[16 * g8:16 * g8 + 16, 0:FW],
                                  in_=idx16w[16 * b:16 * b + 16, :])
                nc.sync.dma_start(out=idx_rep[16 * g8:16 * g8 + 16, FW:2 * FW],
                                  in_=idx16w[64 + 16 * b:64 + 16 * b + 16, :])

            ms = [m0, m1, m2, m3, m4, m5]
            # gather fetches only the 3 used column-pairs (384B) of each
            # 512B col-major unit; slot = 2*col + row, ms order is
            # [r0c0, r0c1, r0c2, r1c0, r1c1, r1c2].
            slot = [0, 2, 4, 1, 3, 5]
            GE = 192              # gathered element: 6 px of 32 ch bf16
            for j in range(NCALL):
                if skip_gather:
                    if b == 0 and j == 0:
                        gt_fixed = singles.tile([P, KB, GE], bf16)
                        nc.vector.memset(gt_fixed[:], 0)
                    gt_t = gt_fixed
                else:
                    gt_t = gpool.tile([P, KB, GE], bf16, tag="gt", name="gt")
                if not skip_gather:
                    # 1024-idx sub-gathers: 64 descs/engine = one packet
                    # (single_packet); more than that fails on HW. The four
                    # sub-gathers go to four SWDGE queues: queue q's desc-gen
                    # runs on Q7 core pair (2q, 2q+1), so the (dominant)
                    # descriptor-generation cost parallelizes 4x.
                    if big_calls:
                        for c2 in range(2):
                            _dma_gather_384(
                                nc.gpsimd,
                                out_ap=gt_t[:, 16 * c2:16 * c2 + 16, :],
                                in_ap=qimg[b][:, 0:GE],
                                idxs_ap=idx_rep[:, 256 * j + 128 * c2:
                                                256 * j + 128 * c2 + 128],
                                num_idxs=NIDX // 2,
                                num_idxs_reg=NIDX // 2,
                                elem_size=GE,
                                elem_step=ELEM,
                                queue_num=2 * c2 + (j % 2),
                                single_packet=False,
                            )
                    else:
                        for c4 in range(4):
                            _dma_gather_384(
                                nc.gpsimd,
                                out_ap=gt_t[:, 8 * c4:8 * c4 + 8, :],
                                in_ap=qimg[b][:, 0:GE],
                                idxs_ap=idx_rep[:, 256 * j + 64 * c4:
                                                256 * j + 64 * c4 + 64],
                                num_idxs=NIDX // 4,
                                num_idxs_reg=NIDX // 4,
                                elem_size=GE,
                                elem_step=ELEM,
                                queue_num=c4,
                            )
                if skip_blend:
                    continue
                csl = slice(KB * j, KB * j + KB)
                ps = psum.tile([P, KB, C], f32, tag="ps", name="ps")
                # per plane: ACT materializes packed bf16 weights, DVE
                # multiplies in 2x bf16 mode, PE accumulates into PSUM.
                for k in range(6):
                    mk = mpool.tile([P, KB, C], bf16, tag=f"mat{k}",
                                    name=f"mat{k}")
                    ACT.activation(
                        out=mk[:],
                        in_=ms[k][:, csl, None].to_broadcast([P, KB, C]),
                        func=mybir.ActivationFunctionType.Copy)
                    pk = ppool.tile([P, KB, C], bf16, tag=f"prod{k}",
                                    name=f"prod{k}")
                    V.tensor_tensor(
                        out=pk[:],
                        in0=gt_t[:, :, 32 * slot[k]:32 * slot[k] + 32],
                        in1=mk[:], op=AF.mult)
                    for hh in range(2):
                        hsl = slice(KB // 2 * hh, KB // 2 * (hh + 1))
                        nc.tensor.matmul(
                            ps[:, hsl], id128[:], pk[:, hsl],
                            start=(k == 0), stop=(k == 5))
                ot = opool.tile([P, KB, C], bf16, tag="ot", name="ot")
                if j % 2 == 0:
                    V.tensor_copy(out=ot[:], in_=ps[:])
                else:
                    ACT.activation(out=ot[:], in_=ps[:],
                                   func=mybir.ActivationFunctionType.Copy)
                nc.sync.dma_start(out=out_d[b, j], in_=ot[:])

    nc.compile()
    return nc


import os
SEQ_LAYOUT = os.environ.get("BILERP_LAYOUT", "seq") == "seq"


def _pix_out(p, q):
    """Pixel id at out-layout position (partition p, column q=32j+kb).

    seq layout: each of the 16 SWDGE channels (idx row r = p%16) walks
    CONSECUTIVE pixels as its column advances -> each DMA engine reads
    near-sequential HBM addresses (row-buffer locality).
    """
    if SEQ_LAYOUT:
        j, kb = q // KB, q % KB
        return (NIDX * j + 1024 * (kb // 8) + 64 * (p % 16)
                + 8 * (kb % 8) + p // 16)
    return 128 * q + p


def _pix_wrapped(p, f):
    """Pixel id at wrapped idx position (partition p, column f)."""
    if SEQ_LAYOUT:
        return (NIDX * (f // 256) + 1024 * ((f % 256) // 64) + 64 * (p % 16)
                + (f % 64) + (HW // 2) * (p // 64))
    return 16 * f + (p % 16) + (HW // 2) * (p // 64)


def _host_tables():
    import jax
    import jax.numpy as jnp

    with jax.default_device(jax.devices('cpu')[0]):
        xs = np.asarray(jnp.linspace(-1.0, 1.0, W), dtype=np.float32)
        ys = np.asarray(jnp.linspace(-1.0, 1.0, H), dtype=np.float32)

    p = np.arange(P)[:, None]
    col = np.arange(Q)[None, :]
    n_o = _pix_out(p, col)                    # out-layout pixel id
    xg_o = xs[n_o % W].astype(np.float32)
    yg_o = ys[n_o // W].astype(np.float32)

    f = np.arange(FW)[None, :]
    i_w = _pix_wrapped(p, f)                  # wrapped pixel id
    xg_w = xs[i_w % W].astype(np.float32)
    yg_w = ys[i_w // W].astype(np.float32)
    return xg_o, yg_o, xg_w, yg_w


def _build_qimg(img_core):
    """img_core [S,256,256,32] f32 -> [S, NU, ELEM] bf16 corner-block table.

    Unit (y, u): rows {y, min(y+1,255)} x cols {2u..2u+3} (clamped), order
    [col, row, ch] (column-major so a 384B gather fetches cols 0-2 of both
    rows; col 3 is never read and only pads the 512B stride).
    """
    img16 = img_core.astype(BF16)       # cast before the 12x expansion
    u = np.arange(W // 2)[:, None]
    k = np.arange(4)[None, :]
    cols = np.minimum(2 * u + k, W - 1)                 # [128,4]
    down = np.concatenate([img16[:, 1:], img16[:, -1:]], axis=1)
    top = img16[:, :, cols, :]                          # [S,256,128,4,32]
    bot = down[:, :, cols, :]
    q = np.stack([top, bot], axis=4)                    # [S,256,128,4,2,32]
    return np.ascontiguousarray(q.reshape(S, NU, ELEM))


def _make_in_maps(image, theta):
    xg_o, yg_o, xg_w, yg_w = _host_tables()
    in_maps = []
    for c in range(N_CORES):
        th_core = theta[c * S:(c + 1) * S]
        q = _build_qimg(image[c * S:(c + 1) * S])
        m = {
            "xg_o": xg_o, "yg_o": yg_o, "xg_w": xg_w, "yg_w": yg_w,
            "id128": np.eye(P, dtype=np.float32).astype(BF16),
            "th_o": np.ascontiguousarray(
                np.tile(th_core.reshape(1, 6 * S), (P, 1)), dtype=np.float32),
            "th_w": np.ascontiguousarray(
                th_core[(np.arange(P) % 64) // 16], dtype=np.float32),
        }
        for b in range(S):
            m[f"qimg{b}"] = q[b]
        in_maps.append(m)
    return in_maps


def kernel(image: np.ndarray, theta: np.ndarray) -> np.ndarray:
    image = np.ascontiguousarray(image, dtype=np.float32)
    theta = np.ascontiguousarray(theta, dtype=np.float32)
    assert image.shape == (B_TOTAL, H, W, C) and theta.shape == (B_TOTAL, 6)

    if "nc" not in _COMPILED:
        _COMPILED["nc"] = _build_nc()
    nc = _COMPILED["nc"]

    in_maps = _make_in_maps(image, theta)
    res = run_bass_kernel_spmd(nc, in_maps, core_ids=list(range(N_CORES)))

    # inverse of _pix_out: where in [NCALL, P, KB] does pixel n live
    p = np.arange(P)[:, None]
    q = np.arange(Q)[None, :]
    n_o = _pix_out(p, q)                       # [P, Q]
    inv = np.empty(HW, np.int64)
    inv[n_o.reshape(-1)] = np.arange(HW)       # flat (p*Q + q) per pixel
    inv_p, inv_q = inv // Q, inv % Q
    inv_j, inv_kb = inv_q // KB, inv_q % KB

    out = np.empty((B_TOTAL, H, W, C), np.float32)
    for c in range(N_CORES):
        raw = res.results[c]["out"]            # [S, NCALL, P, KB, C] bf16
        out[c * S:(c + 1) * S] = (
            raw[:, inv_j, inv_p, inv_kb].reshape(S, H, W, C)
            .astype(np.float32))
    return out
